# revision 1
# baseline (speedup 1.0000x reference)
"""Dice loss on 8 TRN2 NeuronCores.

Strategy (pure data parallel over batch):
  - B=16 samples split 2-per-core across 8 cores.
  - Each core computes, for its 2*21=42 (sample, class) images of 512*512
    pixels, the three spatial reductions the loss needs:
        ta    = sum(y_true)          per image
        tb    = sum(y_pred)          per image
        inter = sum(y_true * y_pred) per image
    Per image ([128, 2048] f32 tile): ScalarE copy-with-accum produces
    ta and tb partials, VectorE tensor_tensor_reduce produces the inter
    partial, all as [128, 1] per-partition sums into a [128, 126] tile.
    A single ones-matmul on TensorE folds the partition dim -> [1, 126].
  - Host gathers the 8 x [1,126] results and finishes the (tiny) [16,21]
    dice/masking arithmetic in numpy.
"""

from contextlib import ExitStack

import numpy as np

import concourse.bass as bass
import concourse.tile as tile
from concourse import bacc, mybir
from concourse.bass_utils import run_bass_kernel_spmd

B, C, H, W = 16, 21, 512, 512
N_CORES = 8
B_LOC = B // N_CORES          # samples per core
IMGS = B_LOC * C              # images per core (42)
P = 128                       # SBUF partitions
F = (H * W) // P              # free elements per partition per image (2048)
ROWS = IMGS * P               # dram rows per input per core (5376)

_COMPILED = None


def _build():
    nc = bacc.Bacc(
        "TRN2", target_bir_lowering=False, debug=False, num_devices=N_CORES
    )
    f32 = mybir.dt.float32
    yt_d = nc.dram_tensor("y_true", [ROWS, F], f32, kind="ExternalInput").ap()
    yp_d = nc.dram_tensor("y_pred", [ROWS, F], f32, kind="ExternalInput").ap()
    out_d = nc.dram_tensor("out", [1, 3 * IMGS], f32, kind="ExternalOutput").ap()

    with tile.TileContext(nc) as tc, ExitStack() as ctx:
        io = ctx.enter_context(tc.tile_pool(name="io", bufs=8))
        small = ctx.enter_context(tc.tile_pool(name="small", bufs=1))
        psum = ctx.enter_context(tc.tile_pool(name="psum", bufs=1, space="PSUM"))

        # columns: [0:IMGS) ta, [IMGS:2*IMGS) tb, [2*IMGS:3*IMGS) inter
        parts = small.tile([P, 3 * IMGS], f32)
        ones = small.tile([P, 1], f32)
        dummy_act = small.tile([P, 1], f32)
        dummy_dve = small.tile([P, 1], f32)
        nc.vector.memset(ones[:], 1.0)

        for i in range(IMGS):
            yt = io.tile([P, F], f32, tag="yt")
            yp = io.tile([P, F], f32, tag="yp")
            nc.sync.dma_start(yt[:], yt_d[i * P : (i + 1) * P, :])
            nc.sync.dma_start(yp[:], yp_d[i * P : (i + 1) * P, :])
            # per-partition sums of y_true / y_pred on ScalarE
            nc.scalar.activation(
                dummy_act.broadcast_to((P, F)),
                yt[:],
                mybir.ActivationFunctionType.Copy,
                accum_out=parts[:, i : i + 1],
            )
            nc.scalar.activation(
                dummy_act.broadcast_to((P, F)),
                yp[:],
                mybir.ActivationFunctionType.Copy,
                accum_out=parts[:, IMGS + i : IMGS + i + 1],
            )
            # per-partition sum of the product on VectorE
            # (TensorScalarPtr with is_scalar_tensor_tensor: out = (in0*1)*in1,
            # accum_out = sum(out); TENSOR_TENSOR_REDUCE faults on this HW path)
            nc.vector.scalar_tensor_tensor(
                out=dummy_dve.broadcast_to((P, F)),
                in0=yt[:],
                scalar=1.0,
                in1=yp[:],
                op0=mybir.AluOpType.mult,
                op1=mybir.AluOpType.mult,
                accum_out=parts[:, 2 * IMGS + i : 2 * IMGS + i + 1],
            )

        acc = psum.tile([1, 3 * IMGS], f32)
        nc.tensor.matmul(acc[:], ones[:], parts[:], start=True, stop=True)
        out_sb = small.tile([1, 3 * IMGS], f32)
        nc.vector.tensor_copy(out_sb[:], acc[:])
        nc.sync.dma_start(out_d[:, :], out_sb[:])

    nc.compile()
    return nc


def _get_compiled():
    global _COMPILED
    if _COMPILED is None:
        _COMPILED = _build()
    return _COMPILED


def run_device_sums(y_pred, y_true, **spmd_kwargs):
    """Run the on-device reductions. Returns (ta, tb, inter) as [B, C] f32
    plus the raw BassKernelResults (for profiling)."""
    nc = _get_compiled()
    yp = np.ascontiguousarray(np.asarray(y_pred, dtype=np.float32)).reshape(
        N_CORES, ROWS, F
    )
    yt = np.ascontiguousarray(np.asarray(y_true, dtype=np.float32)).reshape(
        N_CORES, ROWS, F
    )
    in_maps = [{"y_true": yt[k], "y_pred": yp[k]} for k in range(N_CORES)]
    res = run_bass_kernel_spmd(nc, in_maps, list(range(N_CORES)), **spmd_kwargs)
    per_core = np.stack(
        [np.asarray(res.results[k]["out"]).reshape(3, B_LOC, C) for k in range(N_CORES)]
    )  # [cores, 3, B_LOC, C]
    ta = per_core[:, 0].reshape(B, C)
    tb = per_core[:, 1].reshape(B, C)
    inter = per_core[:, 2].reshape(B, C)
    return ta, tb, inter, res


def _epilogue(ta, tb, inter, bg):
    bg_i = int(bg)
    eps = np.float32(1e-11)
    ta = ta[:, bg_i:]
    tb = tb[:, bg_i:]
    inter = inter[:, bg_i:]
    valid = ta != 0
    dice = np.where(
        valid, np.float32(2.0) * inter / (ta + tb + eps), np.float32(0.0)
    ).astype(np.float32)
    cpt2 = valid.sum(axis=1).astype(np.float32)
    denom = cpt2 - np.float32(bg_i)
    batch_valid = denom != 0
    safe_denom = np.where(batch_valid, denom, np.float32(1.0))
    tmp = np.where(
        batch_valid, dice.sum(axis=1, dtype=np.float32) / safe_denom, np.float32(0.0)
    ).astype(np.float32)
    cpt1 = batch_valid.sum().astype(np.float32)
    loss = np.float32(1.0) - tmp.sum(dtype=np.float32) / max(cpt1, np.float32(1.0))
    result = loss if cpt1 > 0 else np.float32(-1.0)
    return np.asarray(result, dtype=np.float32)


def kernel(y_pred, y_true, bg=0, **_unused):
    ta, tb, inter, _ = run_device_sums(y_pred, y_true)
    return _epilogue(ta, tb, inter, bg)

